# revision 15
# baseline (speedup 1.0000x reference)
"""Dual-key additive attention (nn_Attention_58059367908033) on 8 trn2 NeuronCores.

Reference computation (per batch b, head h, with n = 64*64 = 4096 positions,
d = 128, scale = d**-0.5):
    q  = Wq_h  @ fmap[b]          # [d, n]   (channels-major, "n" = spatial)
    k1 = Wk1_h @ fmap[b]          # [d, n]
    v  = Wv_h  @ fmap[b]          # [d, n]
    k2 = Wk2_h @ x[b]             # [d, n]
    sim  = (scale * q)^T (k1+k2)  # [n, n]  (q rows, key cols)
    attn = softmax(sim, axis=-1)
    out[b, h*d:(h+1)*d] = (attn @ v^T)^T  # [d, n] -> reshape [d, 64, 64]

Sharding: 8 cores = (b in 2) x (h in 2) x (key-half kh in 2).  Each core
computes, for its (b, h) and its 2048-key slice, the *unnormalized*
    U[d, q]   = sum_{k in slice} exp(scale * sim[k, q]) * vT[k, d]
    D[1, q]   = sum_{k in slice} exp(scale * sim[k, q])
streamed flash-attention style (no max subtraction: |scale*sim| is O(1) for
these inputs, fp32 exp is exact-safe).  The host adds the two key-half
partials and divides -- mathematically exact softmax-attention.

Schedule notes (v2):
  - fmap is sent in two column-halves with the core's KEY half first, so the
    value/k1 projections and early query chunks never wait on the full fmap.
    Queries are processed in that permuted column order; the host swaps the
    output halves back (queries are independent, so this is free).
  - ksum chunks are built just-in-time inside the key-chunk loop, interleaved
    with attention for the first two query chunks, so the 8.4 MB x stream
    overlaps real matmuls instead of warmup filler.
  - Remaining query chunks sweep all key tiles back-to-back, keeping the PE
    dense (HAM stays at K=8/8).
"""

import ml_dtypes
import numpy as np

BF16_NP = ml_dtypes.bfloat16

import concourse.bass as bass
import concourse.mybir as mybir
import concourse.tile as tile
from concourse import bacc
from concourse.bass_utils import run_bass_kernel_spmd

HEADS = 2
D = 128          # dim head
C1 = 256         # fmap channels
C2 = 2048        # x channels
N = 4096         # spatial positions (64*64) = queries = keys
KSL = 2048       # keys per core (half)
SCALE = float(D) ** -0.5

F32 = mybir.dt.float32
BF16 = mybir.dt.bfloat16

KC = 4           # key chunks per core (512 keys each)
KT = 4           # k-tiles (128 keys) per key chunk
QC = 8           # query chunks of 512
QW = 512

_COMPILED = {}


def _build_program():
    nc = bacc.Bacc("TRN2", target_bir_lowering=False, debug=False, num_devices=8)

    # ---- DRAM parameters (per-core data, same program on all 8 cores) ----
    # All tensors are pre-transposed on the host to partition-major layout so
    # every DMA moves multi-KB contiguous runs per partition (no strided
    # small-packet transfers).  fmap arrives in two column halves, the core's
    # key half first; x arrives chunked by 512-key groups.
    d_fmapA = nc.dram_tensor("fmapA", [128, 2, KSL], BF16, kind="ExternalInput").ap()
    d_fmapB = nc.dram_tensor("fmapB", [128, 2, KSL], BF16, kind="ExternalInput").ap()
    d_xs = nc.dram_tensor("xs", [128, KC, 16, QW], BF16, kind="ExternalInput").ap()
    d_wqT = nc.dram_tensor("wqT", [128, 2, D], BF16, kind="ExternalInput").ap()
    d_wk1T = nc.dram_tensor("wk1T", [128, 2, D], BF16, kind="ExternalInput").ap()
    d_wvT = nc.dram_tensor("wvT", [128, 2, D], BF16, kind="ExternalInput").ap()
    d_wk2T = nc.dram_tensor("wk2T", [128, 16, D], BF16, kind="ExternalInput").ap()
    # outputs in the permuted query order (key half first); host un-permutes
    d_outU = nc.dram_tensor("outU", [128, N], F32, kind="ExternalOutput").ap()
    d_den = nc.dram_tensor("denom", [1, N], F32, kind="ExternalOutput").ap()

    with tile.TileContext(nc) as tc:
        with (
            tc.tile_pool(name="wts", bufs=1) as wts,
            tc.tile_pool(name="fm", bufs=1) as fm,
            tc.tile_pool(name="big", bufs=1) as big,
            tc.tile_pool(name="xsp", bufs=1) as xsp,
            tc.tile_pool(name="ex", bufs=6) as exp_pool,
            tc.tile_pool(name="acc", bufs=4) as accp,
            tc.tile_pool(name="st", bufs=2) as st,
            tc.tile_pool(name="ps_s", bufs=2, space="PSUM") as ps_s,
            tc.tile_pool(name="ps_o", bufs=3, space="PSUM") as ps_o,
            tc.tile_pool(name="ps_m", bufs=1, space="PSUM") as ps_m,
        ):
            # ---- input DMAs over the 3 DGE queues ----
            # The two HWDGE queues (sync, scalar) carry the critical-path
            # tensors first: fmapA halves, then x0 halves.  gpsimd (SWDGE)
            # carries the small weights plus the x tail.  Scalar-queue DMAs
            # all issue in the prologue, before the first ACTIVATE.
            fmapA = fm.tile([128, 2, KSL], BF16, tag="fmapA")
            fmapB = fm.tile([128, 2, KSL], BF16, tag="fmapB")

            wv = wts.tile([128, 2, D], BF16, tag="wv")
            wk1 = wts.tile([128, 2, D], BF16, tag="wk1")
            wk2 = wts.tile([128, 16, D], BF16, tag="wk2")
            wq = wts.tile([128, 2, D], BF16, tag="wq")

            x_tiles = [xsp.tile([128, 16, QW], BF16, tag="x", name=f"x{i}",
                                bufs=4)
                       for i in range(KC)]

            def load_x_half(kc, half, eng):
                xt = x_tiles[kc]
                hs = slice(half * 8, half * 8 + 8)
                eng.dma_start(xt[:, hs, :], d_xs[:, kc, hs, :])

            # ct-tile halves of fmapA on the two fast queues, weights first
            # on gpsimd, then x0 halves, then the x tail round-robin
            nc.gpsimd.dma_start(wv[:], d_wvT)
            nc.sync.dma_start(fmapA[:, 0, :], d_fmapA[:, 0, :])
            nc.scalar.dma_start(fmapA[:, 1, :], d_fmapA[:, 1, :])
            nc.gpsimd.dma_start(wk1[:], d_wk1T)
            nc.gpsimd.dma_start(wk2[:], d_wk2T)
            nc.gpsimd.dma_start(wq[:], d_wqT)
            load_x_half(0, 0, nc.sync)
            load_x_half(0, 1, nc.scalar)
            load_x_half(1, 0, nc.gpsimd)
            load_x_half(1, 1, nc.scalar)
            load_x_half(2, 0, nc.gpsimd)
            load_x_half(2, 1, nc.sync)
            nc.sync.dma_start(fmapB[:], d_fmapB)
            load_x_half(3, 0, nc.gpsimd)
            load_x_half(3, 1, nc.scalar)

            ones = wts.tile([128, 1], BF16, tag="ones")
            nc.vector.memset(ones[:], 1.0)
            warm = wts.tile([128, 512], BF16, tag="warm")
            nc.vector.memset(warm[:], 0.0)

            def warmup(n):
                for _ in range(n):
                    wps = ps_s.tile([128, QW], F32, tag="ps_sim", name="pswarm")
                    nc.tensor.matmul(wps[:], warm[:, :128], warm[:],
                                     start=True, stop=True)

            # HAM ramp: a few no-dep matmuls before real work arrives
            warmup(6)

            # ---- vT tiles [k=128, d] via fmapA-stationary matmuls ----
            vT = big.tile([128, 16, D], BF16, tag="vT")
            for kt in range(16):
                ps = ps_s.tile([128, D], F32, tag="ps_sim", name="psv")
                ksl = slice(kt * 128, (kt + 1) * 128)
                nc.tensor.matmul(ps[:], fmapA[:, 0, ksl], wv[:, 0, :], start=True, stop=False)
                nc.tensor.matmul(ps[:], fmapA[:, 1, ksl], wv[:, 1, :], start=False, stop=True)
                nc.scalar.copy(vT[:, kt, :], ps[:])

            # ---- q projection (permuted order: key half = chunks 0-3) ----
            q_sb = big.tile([128, QC, QW], BF16, tag="q")

            def build_q(nch):
                src = fmapA if nch < 4 else fmapB
                ps = ps_s.tile([128, QW], F32, tag="ps_sim", name="psq")
                sl = slice((nch % 4) * QW, (nch % 4 + 1) * QW)
                nc.tensor.matmul(ps[:], wq[:, 0, :], src[:, 0, sl], start=True, stop=False)
                nc.tensor.matmul(ps[:], wq[:, 1, :], src[:, 1, sl], start=False, stop=True)
                nc.vector.tensor_copy(q_sb[:, nch, :], ps[:])

            for nch in range(4):
                build_q(nch)
            warmup(2)

            ksum = big.tile([128, KSL], BF16, tag="ksum")

            def build_ksum(kc):
                kps = ps_m.tile([128, QW], F32, tag="ps_misc", name="kps")
                sl = slice(kc * QW, (kc + 1) * QW)
                nc.tensor.matmul(kps[:], wk1[:, 0, :], fmapA[:, 0, sl],
                                 start=True, stop=False)
                nc.tensor.matmul(kps[:], wk1[:, 1, :], fmapA[:, 1, sl],
                                 start=False, stop=False)
                xt = x_tiles[kc]
                for ct in range(16):
                    nc.tensor.matmul(kps[:], wk2[:, ct, :], xt[:, ct, :],
                                     start=False, stop=(ct == 15))
                nc.vector.tensor_copy(ksum[:, sl], kps[:])

            # ---- attention inner step -------------------------------------
            from collections import deque
            pendq = deque()
            ops = {}
            accs = {}

            def attn(qc, kc):
                if kc == 0:
                    ops[qc] = ps_o.tile([128, QW], F32, tag="ps_out",
                                        name=f"ops{qc}")
                    # two-lane denominator accumulator: one flat [128,2,512]
                    # bf16 add per exp tile
                    accs[qc] = accp.tile([128, 2, QW], BF16, tag="dacc",
                                         name=f"acc{qc}")
                ops0 = ops[qc]
                acc = accs[qc]
                for sg in range(KT // 2):
                    sps = ps_s.tile([128, 2, QW], F32, tag="ps_sim")
                    et = exp_pool.tile([128, 2, QW], BF16, tag="exp")
                    for j in range(2):
                        kk = kc * KT + sg * 2 + j
                        nc.tensor.matmul(
                            sps[:, j, :],
                            ksum[:, kk * 128:(kk + 1) * 128], q_sb[:, qc, :],
                            start=True, stop=True)
                    nc.scalar.activation(et[:], sps[:],
                                         mybir.ActivationFunctionType.Exp,
                                         scale=SCALE)
                    if len(pendq) >= 1:
                        pendq.popleft()()

                    first = (kc == 0 and sg == 0)
                    last = (kc == KC - 1 and sg == KT // 2 - 1)

                    def _pend(qc0=qc, kc0=kc, sg0=sg, et0=et, ops0=ops0,
                              acc0=acc, first=first, last=last):
                        for j in range(2):
                            kk = kc0 * KT + sg0 * 2 + j
                            nc.tensor.matmul(ops0[:], vT[:, kk, :],
                                             et0[:, j, :],
                                             start=(first and j == 0),
                                             stop=(last and j == 1))
                        if first:
                            nc.vector.tensor_copy(acc0[:], et0[:])
                        else:
                            nc.vector.tensor_add(acc0[:], acc0[:], et0[:])
                    pendq.append(_pend)

                    if last:
                        # denominator + store finisher runs one pipeline slot
                        # later so the PE never waits on the DVE add chain
                        def _fin(qc0=qc, ops0=ops0, acc0=acc):
                            qsl0 = slice(qc0 * QW, (qc0 + 1) * QW)
                            dps = ps_m.tile([1, QW], F32, tag="ps_misc")
                            nc.tensor.matmul(dps[:], ones[:], acc0[:, 0, :],
                                             start=True, stop=False)
                            nc.tensor.matmul(dps[:], ones[:], acc0[:, 1, :],
                                             start=False, stop=True)
                            den_st = st.tile([1, QW], F32, tag="den_st")
                            nc.vector.tensor_copy(den_st[:], dps[:])
                            nc.sync.dma_start(d_den[:, qsl0], den_st[:])
                            out_st = st.tile([128, QW], F32, tag="out_st")
                            nc.vector.tensor_copy(out_st[:], ops0[:])
                            nc.sync.dma_start(d_outU[:, qsl0], out_st[:])
                        pendq.append(_fin)

            # ---- schedule: ksum built just-in-time with qc 0,1 woven in ----
            build_ksum(0)
            attn(0, 0)
            attn(1, 0)
            build_ksum(1)
            attn(0, 1)
            attn(1, 1)
            build_q(4)
            build_ksum(2)
            attn(0, 2)
            attn(1, 2)
            build_ksum(3)
            attn(0, 3)
            attn(1, 3)
            for qc in range(2, QC):
                # feed q projections into the ACT-gated sweep to keep PE busy
                if qc + 3 < QC:
                    build_q(qc + 3)
                for kc in range(KC):
                    attn(qc, kc)
            while pendq:
                pendq.popleft()()

    nc.compile()
    return nc


def _prep_inputs(fmap, x, Wqkv, Wk2):
    """Host-side slicing: per-core input dicts. Core c = b*4 + h*2 + kh."""
    fmap = np.ascontiguousarray(fmap, dtype=np.float32)
    x = np.ascontiguousarray(x, dtype=np.float32)
    Wqkv = np.ascontiguousarray(Wqkv, dtype=np.float32)
    Wk2 = np.ascontiguousarray(Wk2, dtype=np.float32)

    in_maps = []
    for c in range(8):
        b, h, kh = c // 4, (c // 2) % 2, c % 2
        fb = fmap[b].reshape(C1, N)
        xb = x[b].reshape(C2, N)
        ks = slice(kh * KSL, (kh + 1) * KSL)
        other = slice((1 - kh) * KSL, (2 - kh) * KSL)
        wq = Wqkv[h * D:(h + 1) * D]              # [128, 256]
        wk1 = Wqkv[C1 + h * D:C1 + (h + 1) * D]
        wv = Wqkv[2 * C1 + h * D:2 * C1 + (h + 1) * D]
        wk2 = Wk2[h * D:(h + 1) * D]              # [128, 2048]
        # partition-major layouts: [128, t, n] so DMAs are contiguous per
        # partition; x additionally pre-chunked by 512-key groups
        in_maps.append({
            "fmapA": np.ascontiguousarray(
                fb[:, ks].reshape(2, 128, KSL).transpose(1, 0, 2)).astype(BF16_NP),
            "fmapB": np.ascontiguousarray(
                fb[:, other].reshape(2, 128, KSL).transpose(1, 0, 2)).astype(BF16_NP),
            "xs": np.ascontiguousarray(
                xb[:, ks].reshape(16, 128, KC, QW).transpose(1, 2, 0, 3)).astype(BF16_NP),
            "wqT": np.ascontiguousarray(
                wq.T.reshape(2, 128, D).transpose(1, 0, 2)).astype(BF16_NP),
            "wk1T": np.ascontiguousarray(
                wk1.T.reshape(2, 128, D).transpose(1, 0, 2)).astype(BF16_NP),
            "wvT": np.ascontiguousarray(
                wv.T.reshape(2, 128, D).transpose(1, 0, 2)).astype(BF16_NP),
            "wk2T": np.ascontiguousarray(
                wk2.T.reshape(16, 128, D).transpose(1, 0, 2)).astype(BF16_NP),
        })
    return in_maps


def _combine(results):
    """Host epilogue: un-permute query columns, add key-half partials,
    normalize, assemble output."""
    out = np.empty((2, HEADS * D, 64, 64), dtype=np.float32)
    for b in range(2):
        for h in range(2):
            U = np.empty((D, N), dtype=np.float32)
            Dn = np.empty((1, N), dtype=np.float32)
            for kh in range(2):
                r = results[b * 4 + h * 2 + kh]
                # core kh processed queries in order [kh half, other half]
                cols = np.r_[kh * KSL:(kh + 1) * KSL, (1 - kh) * KSL:(2 - kh) * KSL]
                if kh == 0:
                    U[:, cols] = r["outU"]
                    Dn[:, cols] = r["denom"]
                else:
                    U[:, cols] += r["outU"]
                    Dn[:, cols] += r["denom"]
            out[b, h * D:(h + 1) * D] = (U / Dn).reshape(D, 64, 64)
    return out


def run_on_device(in_maps, trace=False, **kw):
    if "nc" not in _COMPILED:
        _COMPILED["nc"] = _build_program()
    return run_bass_kernel_spmd(_COMPILED["nc"], in_maps, list(range(8)),
                                trace=trace, **kw)


def kernel(fmap, x, Wqkv, Wk2):
    in_maps = _prep_inputs(fmap, x, Wqkv, Wk2)
    res = run_on_device(in_maps)
    return _combine(res.results)


# revision 17
# speedup vs baseline: 1.0141x; 1.0141x over previous
"""Dual-key additive attention (nn_Attention_58059367908033) on 8 trn2 NeuronCores.

Reference computation (per batch b, head h, with n = 64*64 = 4096 positions,
d = 128, scale = d**-0.5):
    q  = Wq_h  @ fmap[b]          # [d, n]   (channels-major, "n" = spatial)
    k1 = Wk1_h @ fmap[b]          # [d, n]
    v  = Wv_h  @ fmap[b]          # [d, n]
    k2 = Wk2_h @ x[b]             # [d, n]
    sim  = (scale * q)^T (k1+k2)  # [n, n]  (q rows, key cols)
    attn = softmax(sim, axis=-1)
    out[b, h*d:(h+1)*d] = (attn @ v^T)^T  # [d, n] -> reshape [d, 64, 64]

Sharding: 8 cores = (b in 2) x (h in 2) x (key-half kh in 2).  Each core
computes, for its (b, h) and its 2048-key slice, the *unnormalized*
    U[d, q]   = sum_{k in slice} exp(scale * sim[k, q]) * vT[k, d]
    D[1, q]   = sum_{k in slice} exp(scale * sim[k, q])
streamed flash-attention style (no max subtraction: |scale*sim| is O(1) for
these inputs, fp32 exp is exact-safe).  The host adds the two key-half
partials and divides -- mathematically exact softmax-attention.

Schedule notes (v2):
  - fmap is sent in two column-halves with the core's KEY half first, so the
    value/k1 projections and early query chunks never wait on the full fmap.
    Queries are processed in that permuted column order; the host swaps the
    output halves back (queries are independent, so this is free).
  - ksum chunks are built just-in-time inside the key-chunk loop, interleaved
    with attention for the first two query chunks, so the 8.4 MB x stream
    overlaps real matmuls instead of warmup filler.
  - Remaining query chunks sweep all key tiles back-to-back, keeping the PE
    dense (HAM stays at K=8/8).
"""

import ml_dtypes
import numpy as np

BF16_NP = ml_dtypes.bfloat16

import concourse.bass as bass
import concourse.mybir as mybir
import concourse.tile as tile
from concourse import bacc
from concourse.bass_utils import run_bass_kernel_spmd

HEADS = 2
D = 128          # dim head
C1 = 256         # fmap channels
C2 = 2048        # x channels
N = 4096         # spatial positions (64*64) = queries = keys
KSL = 2048       # keys per core (half)
SCALE = float(D) ** -0.5

F32 = mybir.dt.float32
BF16 = mybir.dt.bfloat16

KC = 4           # key chunks per core (512 keys each)
KT = 4           # k-tiles (128 keys) per key chunk
QC = 8           # query chunks of 512
QW = 512

_COMPILED = {}


def _build_program():
    nc = bacc.Bacc("TRN2", target_bir_lowering=False, debug=False, num_devices=8)

    # ---- DRAM parameters (per-core data, same program on all 8 cores) ----
    # All tensors are pre-transposed on the host to partition-major layout so
    # every DMA moves multi-KB contiguous runs per partition (no strided
    # small-packet transfers).  fmap arrives in two column halves, the core's
    # key half first; x arrives chunked by 512-key groups.
    d_fmapA = nc.dram_tensor("fmapA", [128, 2, KSL], BF16, kind="ExternalInput").ap()
    d_fmapB = nc.dram_tensor("fmapB", [128, 2, KSL], BF16, kind="ExternalInput").ap()
    d_xs = nc.dram_tensor("xs", [128, KC, 16, QW], BF16, kind="ExternalInput").ap()
    d_wqT = nc.dram_tensor("wqT", [128, 2, D], BF16, kind="ExternalInput").ap()
    d_wk1T = nc.dram_tensor("wk1T", [128, 2, D], BF16, kind="ExternalInput").ap()
    d_wvT = nc.dram_tensor("wvT", [128, 2, D], BF16, kind="ExternalInput").ap()
    d_wk2T = nc.dram_tensor("wk2T", [128, 16, D], BF16, kind="ExternalInput").ap()
    # outputs in the permuted query order (key half first); host un-permutes
    d_outU = nc.dram_tensor("outU", [128, N], F32, kind="ExternalOutput").ap()
    d_den = nc.dram_tensor("denom", [1, N], F32, kind="ExternalOutput").ap()

    with tile.TileContext(nc) as tc:
        with (
            tc.tile_pool(name="wts", bufs=1) as wts,
            tc.tile_pool(name="fm", bufs=1) as fm,
            tc.tile_pool(name="big", bufs=1) as big,
            tc.tile_pool(name="xsp", bufs=1) as xsp,
            tc.tile_pool(name="ex", bufs=6) as exp_pool,
            tc.tile_pool(name="acc", bufs=4) as accp,
            tc.tile_pool(name="st", bufs=2) as st,
            tc.tile_pool(name="ps_s", bufs=2, space="PSUM") as ps_s,
            tc.tile_pool(name="ps_o", bufs=3, space="PSUM") as ps_o,
            tc.tile_pool(name="ps_m", bufs=1, space="PSUM") as ps_m,
        ):
            # ---- input DMAs over the 3 DGE queues ----
            # The two HWDGE queues (sync, scalar) carry the critical-path
            # tensors first: fmapA halves, then x0 halves.  gpsimd (SWDGE)
            # carries the small weights plus the x tail.  Scalar-queue DMAs
            # all issue in the prologue, before the first ACTIVATE.
            fmapA = fm.tile([128, 2, KSL], BF16, tag="fmapA")
            fmapB = fm.tile([128, 2, KSL], BF16, tag="fmapB")

            wv = wts.tile([128, 2, D], BF16, tag="wv")
            wk1 = wts.tile([128, 2, D], BF16, tag="wk1")
            wk2 = wts.tile([128, 16, D], BF16, tag="wk2")
            wq = wts.tile([128, 2, D], BF16, tag="wq")

            x_tiles = [xsp.tile([128, 16, QW], BF16, tag="x", name=f"x{i}",
                                bufs=4)
                       for i in range(KC)]

            def load_x_half(kc, half, eng):
                xt = x_tiles[kc]
                hs = slice(half * 8, half * 8 + 8)
                eng.dma_start(xt[:, hs, :], d_xs[:, kc, hs, :])

            # Deadline-driven spread.  The SWDGE (gpsimd) queue starts late
            # and runs ~110 GB/s, so it only carries the late x tail; both
            # HWDGE queues carry the critical path: fmapA halves, weights,
            # x0, then the rest by need-time.
            nc.sync.dma_start(fmapA[:, 0, :], d_fmapA[:, 0, :])
            nc.scalar.dma_start(fmapA[:, 1, :], d_fmapA[:, 1, :])
            nc.scalar.dma_start(wv[:], d_wvT)
            nc.scalar.dma_start(wk2[:], d_wk2T)
            nc.sync.dma_start(wk1[:], d_wk1T)
            nc.sync.dma_start(wq[:], d_wqT)
            load_x_half(0, 0, nc.sync)
            load_x_half(0, 1, nc.scalar)
            load_x_half(1, 0, nc.gpsimd)
            load_x_half(1, 1, nc.scalar)
            nc.sync.dma_start(fmapB[:], d_fmapB)
            load_x_half(2, 0, nc.gpsimd)
            load_x_half(2, 1, nc.scalar)
            load_x_half(3, 0, nc.gpsimd)
            load_x_half(3, 1, nc.scalar)

            ones = wts.tile([128, 1], BF16, tag="ones")
            nc.vector.memset(ones[:], 1.0)
            warm = wts.tile([128, 512], BF16, tag="warm")
            nc.vector.memset(warm[:], 0.0)

            def warmup(n):
                for _ in range(n):
                    wps = ps_s.tile([128, QW], F32, tag="ps_sim", name="pswarm")
                    nc.tensor.matmul(wps[:], warm[:, :128], warm[:],
                                     start=True, stop=True)

            # HAM ramp: a few no-dep matmuls before real work arrives
            warmup(6)

            # ---- vT tiles [k=128, d] via fmapA-stationary matmuls ----
            vT = big.tile([128, 16, D], BF16, tag="vT")
            for kt in range(16):
                ps = ps_s.tile([128, D], F32, tag="ps_sim", name="psv")
                ksl = slice(kt * 128, (kt + 1) * 128)
                nc.tensor.matmul(ps[:], fmapA[:, 0, ksl], wv[:, 0, :], start=True, stop=False)
                nc.tensor.matmul(ps[:], fmapA[:, 1, ksl], wv[:, 1, :], start=False, stop=True)
                nc.vector.tensor_copy(vT[:, kt, :], ps[:])

            # ---- q projection (permuted order: key half = chunks 0-3) ----
            q_sb = big.tile([128, QC, QW], BF16, tag="q")

            def build_q(nch):
                src = fmapA if nch < 4 else fmapB
                ps = ps_s.tile([128, QW], F32, tag="ps_sim", name="psq")
                sl = slice((nch % 4) * QW, (nch % 4 + 1) * QW)
                nc.tensor.matmul(ps[:], wq[:, 0, :], src[:, 0, sl], start=True, stop=False)
                nc.tensor.matmul(ps[:], wq[:, 1, :], src[:, 1, sl], start=False, stop=True)
                nc.vector.tensor_copy(q_sb[:, nch, :], ps[:])

            for nch in range(4):
                build_q(nch)
            warmup(2)

            ksum = big.tile([128, KSL], BF16, tag="ksum")

            def build_ksum(kc):
                kps = ps_m.tile([128, QW], F32, tag="ps_misc", name="kps")
                sl = slice(kc * QW, (kc + 1) * QW)
                nc.tensor.matmul(kps[:], wk1[:, 0, :], fmapA[:, 0, sl],
                                 start=True, stop=False)
                nc.tensor.matmul(kps[:], wk1[:, 1, :], fmapA[:, 1, sl],
                                 start=False, stop=False)
                xt = x_tiles[kc]
                for ct in range(16):
                    nc.tensor.matmul(kps[:], wk2[:, ct, :], xt[:, ct, :],
                                     start=False, stop=(ct == 15))
                nc.vector.tensor_copy(ksum[:, sl], kps[:])

            # ---- attention inner step -------------------------------------
            from collections import deque
            pendq = deque()
            ops = {}
            accs = {}

            def attn(qc, kc):
                if kc == 0:
                    ops[qc] = ps_o.tile([128, QW], F32, tag="ps_out",
                                        name=f"ops{qc}")
                    # two-lane denominator accumulator: one flat [128,2,512]
                    # bf16 add per exp tile
                    accs[qc] = accp.tile([128, 2, QW], BF16, tag="dacc",
                                         name=f"acc{qc}")
                ops0 = ops[qc]
                acc = accs[qc]
                for sg in range(KT // 2):
                    sps = ps_s.tile([128, 2, QW], F32, tag="ps_sim")
                    et = exp_pool.tile([128, 2, QW], BF16, tag="exp")
                    for j in range(2):
                        kk = kc * KT + sg * 2 + j
                        nc.tensor.matmul(
                            sps[:, j, :],
                            ksum[:, kk * 128:(kk + 1) * 128], q_sb[:, qc, :],
                            start=True, stop=True)
                    nc.scalar.activation(et[:], sps[:],
                                         mybir.ActivationFunctionType.Exp,
                                         scale=SCALE)
                    if len(pendq) >= 1:
                        pendq.popleft()()

                    first = (kc == 0 and sg == 0)
                    last = (kc == KC - 1 and sg == KT // 2 - 1)

                    def _pend(qc0=qc, kc0=kc, sg0=sg, et0=et, ops0=ops0,
                              acc0=acc, first=first, last=last):
                        for j in range(2):
                            kk = kc0 * KT + sg0 * 2 + j
                            nc.tensor.matmul(ops0[:], vT[:, kk, :],
                                             et0[:, j, :],
                                             start=(first and j == 0),
                                             stop=(last and j == 1))
                        if first:
                            nc.vector.tensor_copy(acc0[:], et0[:])
                        else:
                            nc.vector.tensor_add(acc0[:], acc0[:], et0[:])
                    pendq.append(_pend)

                    if last:
                        # denominator + store finisher runs one pipeline slot
                        # later so the PE never waits on the DVE add chain
                        def _fin(qc0=qc, ops0=ops0, acc0=acc):
                            qsl0 = slice(qc0 * QW, (qc0 + 1) * QW)
                            dps = ps_m.tile([1, QW], F32, tag="ps_misc")
                            nc.tensor.matmul(dps[:], ones[:], acc0[:, 0, :],
                                             start=True, stop=False)
                            nc.tensor.matmul(dps[:], ones[:], acc0[:, 1, :],
                                             start=False, stop=True)
                            den_st = st.tile([1, QW], F32, tag="den_st")
                            nc.vector.tensor_copy(den_st[:], dps[:])
                            nc.sync.dma_start(d_den[:, qsl0], den_st[:])
                            out_st = st.tile([128, QW], F32, tag="out_st")
                            nc.vector.tensor_copy(out_st[:], ops0[:])
                            nc.sync.dma_start(d_outU[:, qsl0], out_st[:])
                        pendq.append(_fin)

            # ---- schedule: ksum built just-in-time with qc 0,1 woven in ----
            build_ksum(0)
            attn(0, 0)
            attn(1, 0)
            build_ksum(1)
            attn(0, 1)
            attn(1, 1)
            build_q(4)
            build_ksum(2)
            attn(0, 2)
            attn(1, 2)
            build_ksum(3)
            attn(0, 3)
            attn(1, 3)
            for qc in range(2, QC):
                # feed q projections into the ACT-gated sweep to keep PE busy
                if qc + 3 < QC:
                    build_q(qc + 3)
                for kc in range(KC):
                    attn(qc, kc)
            while pendq:
                pendq.popleft()()

    nc.compile()
    return nc


def _prep_inputs(fmap, x, Wqkv, Wk2):
    """Host-side slicing: per-core input dicts. Core c = b*4 + h*2 + kh."""
    fmap = np.ascontiguousarray(fmap, dtype=np.float32)
    x = np.ascontiguousarray(x, dtype=np.float32)
    Wqkv = np.ascontiguousarray(Wqkv, dtype=np.float32)
    Wk2 = np.ascontiguousarray(Wk2, dtype=np.float32)

    in_maps = []
    for c in range(8):
        b, h, kh = c // 4, (c // 2) % 2, c % 2
        fb = fmap[b].reshape(C1, N)
        xb = x[b].reshape(C2, N)
        ks = slice(kh * KSL, (kh + 1) * KSL)
        other = slice((1 - kh) * KSL, (2 - kh) * KSL)
        wq = Wqkv[h * D:(h + 1) * D]              # [128, 256]
        wk1 = Wqkv[C1 + h * D:C1 + (h + 1) * D]
        wv = Wqkv[2 * C1 + h * D:2 * C1 + (h + 1) * D]
        wk2 = Wk2[h * D:(h + 1) * D]              # [128, 2048]
        # partition-major layouts: [128, t, n] so DMAs are contiguous per
        # partition; x additionally pre-chunked by 512-key groups
        in_maps.append({
            "fmapA": np.ascontiguousarray(
                fb[:, ks].reshape(2, 128, KSL).transpose(1, 0, 2)).astype(BF16_NP),
            "fmapB": np.ascontiguousarray(
                fb[:, other].reshape(2, 128, KSL).transpose(1, 0, 2)).astype(BF16_NP),
            "xs": np.ascontiguousarray(
                xb[:, ks].reshape(16, 128, KC, QW).transpose(1, 2, 0, 3)).astype(BF16_NP),
            "wqT": np.ascontiguousarray(
                wq.T.reshape(2, 128, D).transpose(1, 0, 2)).astype(BF16_NP),
            "wk1T": np.ascontiguousarray(
                wk1.T.reshape(2, 128, D).transpose(1, 0, 2)).astype(BF16_NP),
            "wvT": np.ascontiguousarray(
                wv.T.reshape(2, 128, D).transpose(1, 0, 2)).astype(BF16_NP),
            "wk2T": np.ascontiguousarray(
                wk2.T.reshape(16, 128, D).transpose(1, 0, 2)).astype(BF16_NP),
        })
    return in_maps


def _combine(results):
    """Host epilogue: un-permute query columns, add key-half partials,
    normalize, assemble output."""
    out = np.empty((2, HEADS * D, 64, 64), dtype=np.float32)
    for b in range(2):
        for h in range(2):
            U = np.empty((D, N), dtype=np.float32)
            Dn = np.empty((1, N), dtype=np.float32)
            for kh in range(2):
                r = results[b * 4 + h * 2 + kh]
                # core kh processed queries in order [kh half, other half]
                cols = np.r_[kh * KSL:(kh + 1) * KSL, (1 - kh) * KSL:(2 - kh) * KSL]
                if kh == 0:
                    U[:, cols] = r["outU"]
                    Dn[:, cols] = r["denom"]
                else:
                    U[:, cols] += r["outU"]
                    Dn[:, cols] += r["denom"]
            out[b, h * D:(h + 1) * D] = (U / Dn).reshape(D, 64, 64)
    return out


def run_on_device(in_maps, trace=False, **kw):
    if "nc" not in _COMPILED:
        _COMPILED["nc"] = _build_program()
    return run_bass_kernel_spmd(_COMPILED["nc"], in_maps, list(range(8)),
                                trace=trace, **kw)


def kernel(fmap, x, Wqkv, Wk2):
    in_maps = _prep_inputs(fmap, x, Wqkv, Wk2)
    res = run_on_device(in_maps)
    return _combine(res.results)


# revision 18
# speedup vs baseline: 1.0418x; 1.0274x over previous
"""Dual-key additive attention (nn_Attention_58059367908033) on 8 trn2 NeuronCores.

Reference computation (per batch b, head h, with n = 64*64 = 4096 positions,
d = 128, scale = d**-0.5):
    q  = Wq_h  @ fmap[b]          # [d, n]   (channels-major, "n" = spatial)
    k1 = Wk1_h @ fmap[b]          # [d, n]
    v  = Wv_h  @ fmap[b]          # [d, n]
    k2 = Wk2_h @ x[b]             # [d, n]
    sim  = (scale * q)^T (k1+k2)  # [n, n]  (q rows, key cols)
    attn = softmax(sim, axis=-1)
    out[b, h*d:(h+1)*d] = (attn @ v^T)^T  # [d, n] -> reshape [d, 64, 64]

Sharding: 8 cores = (b in 2) x (h in 2) x (key-half kh in 2).  Each core
computes, for its (b, h) and its 2048-key slice, the *unnormalized*
    U[d, q]   = sum_{k in slice} exp(scale * sim[k, q]) * vT[k, d]
    D[1, q]   = sum_{k in slice} exp(scale * sim[k, q])
streamed flash-attention style (no max subtraction: |scale*sim| is O(1) for
these inputs, fp32 exp is exact-safe).  The host adds the two key-half
partials and divides -- mathematically exact softmax-attention.

Schedule notes (v2):
  - fmap is sent in two column-halves with the core's KEY half first, so the
    value/k1 projections and early query chunks never wait on the full fmap.
    Queries are processed in that permuted column order; the host swaps the
    output halves back (queries are independent, so this is free).
  - ksum chunks are built just-in-time inside the key-chunk loop, interleaved
    with attention for the first two query chunks, so the 8.4 MB x stream
    overlaps real matmuls instead of warmup filler.
  - Remaining query chunks sweep all key tiles back-to-back, keeping the PE
    dense (HAM stays at K=8/8).
"""

import ml_dtypes
import numpy as np

BF16_NP = ml_dtypes.bfloat16

import concourse.bass as bass
import concourse.mybir as mybir
import concourse.tile as tile
from concourse import bacc
from concourse.bass_utils import run_bass_kernel_spmd

HEADS = 2
D = 128          # dim head
C1 = 256         # fmap channels
C2 = 2048        # x channels
N = 4096         # spatial positions (64*64) = queries = keys
KSL = 2048       # keys per core (half)
SCALE = float(D) ** -0.5

F32 = mybir.dt.float32
BF16 = mybir.dt.bfloat16

KC = 4           # key chunks per core (512 keys each)
KT = 4           # k-tiles (128 keys) per key chunk
QC = 8           # query chunks of 512
QW = 512

_COMPILED = {}


def _build_program():
    nc = bacc.Bacc("TRN2", target_bir_lowering=False, debug=False, num_devices=8)

    # ---- DRAM parameters (per-core data, same program on all 8 cores) ----
    # All tensors are pre-transposed on the host to partition-major layout so
    # every DMA moves multi-KB contiguous runs per partition (no strided
    # small-packet transfers).  fmap arrives in two column halves, the core's
    # key half first; x arrives chunked by 512-key groups.
    d_fmapA = nc.dram_tensor("fmapA", [128, 2, KSL], BF16, kind="ExternalInput").ap()
    d_fmapB = nc.dram_tensor("fmapB", [128, 2, KSL], BF16, kind="ExternalInput").ap()
    d_xs = nc.dram_tensor("xs", [128, KC, 16, QW], BF16, kind="ExternalInput").ap()
    d_wqT = nc.dram_tensor("wqT", [128, 2, D], BF16, kind="ExternalInput").ap()
    d_wk1T = nc.dram_tensor("wk1T", [128, 2, D], BF16, kind="ExternalInput").ap()
    d_wvT = nc.dram_tensor("wvT", [128, 2, D], BF16, kind="ExternalInput").ap()
    d_wk2T = nc.dram_tensor("wk2T", [128, 16, D], BF16, kind="ExternalInput").ap()
    # outputs in the permuted query order (key half first); host un-permutes
    d_outU = nc.dram_tensor("outU", [128, N], F32, kind="ExternalOutput").ap()
    d_den = nc.dram_tensor("denom", [1, N], F32, kind="ExternalOutput").ap()

    with tile.TileContext(nc) as tc:
        with (
            tc.tile_pool(name="wts", bufs=1) as wts,
            tc.tile_pool(name="fm", bufs=1) as fm,
            tc.tile_pool(name="big", bufs=1) as big,
            tc.tile_pool(name="xsp", bufs=1) as xsp,
            tc.tile_pool(name="ex", bufs=6) as exp_pool,
            tc.tile_pool(name="acc", bufs=4) as accp,
            tc.tile_pool(name="st", bufs=2) as st,
            tc.tile_pool(name="ps_s", bufs=2, space="PSUM") as ps_s,
            tc.tile_pool(name="ps_o", bufs=3, space="PSUM") as ps_o,
            tc.tile_pool(name="ps_m", bufs=1, space="PSUM") as ps_m,
        ):
            # ---- input DMAs over the 3 DGE queues ----
            # The two HWDGE queues (sync, scalar) carry the critical-path
            # tensors first: fmapA halves, then x0 halves.  gpsimd (SWDGE)
            # carries the small weights plus the x tail.  Scalar-queue DMAs
            # all issue in the prologue, before the first ACTIVATE.
            fmapA = fm.tile([128, 2, KSL], BF16, tag="fmapA")
            fmapB = fm.tile([128, 2, KSL], BF16, tag="fmapB")

            wv = wts.tile([128, 2, D], BF16, tag="wv")
            wk1 = wts.tile([128, 2, D], BF16, tag="wk1")
            wk2 = wts.tile([128, 16, D], BF16, tag="wk2")
            wq = wts.tile([128, 2, D], BF16, tag="wq")

            x_tiles = [xsp.tile([128, 16, QW], BF16, tag="x", name=f"x{i}",
                                bufs=4)
                       for i in range(KC)]

            def load_x_half(kc, half, eng):
                xt = x_tiles[kc]
                hs = slice(half * 8, half * 8 + 8)
                eng.dma_start(xt[:, hs, :], d_xs[:, kc, hs, :])

            # Deadline-driven spread.  The two HWDGE queues carry the
            # critical path (fmapA halves, weights, x0); the gpsimd queue's
            # x-tail DMAs are GATED behind fmapA / x0 completion via dummy
            # dependency ops so they don't steal HBM bandwidth from the
            # critical transfers (priority inversion).
            nc.sync.dma_start(fmapA[:, 0, :], d_fmapA[:, 0, :])
            nc.scalar.dma_start(fmapA[:, 1, :], d_fmapA[:, 1, :])
            nc.scalar.dma_start(wv[:], d_wvT)
            nc.scalar.dma_start(wk2[:], d_wk2T)
            nc.sync.dma_start(wk1[:], d_wk1T)
            nc.sync.dma_start(wq[:], d_wqT)
            load_x_half(0, 0, nc.sync)
            load_x_half(0, 1, nc.scalar)
            gate = wts.tile([1, 8], BF16, tag="gate")
            nc.gpsimd.tensor_copy(gate[:], fmapA[:1, 0, :8])
            load_x_half(1, 0, nc.gpsimd)
            load_x_half(1, 1, nc.sync)
            nc.gpsimd.tensor_copy(gate[:], x_tiles[0][:1, 0, :8])
            load_x_half(2, 0, nc.gpsimd)
            load_x_half(2, 1, nc.scalar)
            nc.sync.dma_start(fmapB[:], d_fmapB)
            load_x_half(3, 0, nc.gpsimd)
            load_x_half(3, 1, nc.scalar)

            ones = wts.tile([128, 1], BF16, tag="ones")
            nc.vector.memset(ones[:], 1.0)
            warm = wts.tile([128, 512], BF16, tag="warm")
            nc.vector.memset(warm[:], 0.0)

            def warmup(n):
                for _ in range(n):
                    wps = ps_s.tile([128, QW], F32, tag="ps_sim", name="pswarm")
                    nc.tensor.matmul(wps[:], warm[:, :128], warm[:],
                                     start=True, stop=True)

            # HAM ramp: a few no-dep matmuls before real work arrives
            warmup(6)

            # ---- vT tiles [k=128, d] via fmapA-stationary matmuls ----
            vT = big.tile([128, 16, D], BF16, tag="vT")
            for kt in range(16):
                ps = ps_s.tile([128, D], F32, tag="ps_sim", name="psv")
                ksl = slice(kt * 128, (kt + 1) * 128)
                nc.tensor.matmul(ps[:], fmapA[:, 0, ksl], wv[:, 0, :], start=True, stop=False)
                nc.tensor.matmul(ps[:], fmapA[:, 1, ksl], wv[:, 1, :], start=False, stop=True)
                nc.vector.tensor_copy(vT[:, kt, :], ps[:])

            # ---- q projection (permuted order: key half = chunks 0-3) ----
            q_sb = big.tile([128, QC, QW], BF16, tag="q")

            def build_q(nch):
                src = fmapA if nch < 4 else fmapB
                ps = ps_s.tile([128, QW], F32, tag="ps_sim", name="psq")
                sl = slice((nch % 4) * QW, (nch % 4 + 1) * QW)
                nc.tensor.matmul(ps[:], wq[:, 0, :], src[:, 0, sl], start=True, stop=False)
                nc.tensor.matmul(ps[:], wq[:, 1, :], src[:, 1, sl], start=False, stop=True)
                nc.vector.tensor_copy(q_sb[:, nch, :], ps[:])

            for nch in range(4):
                build_q(nch)
            warmup(2)

            ksum = big.tile([128, KSL], BF16, tag="ksum")

            def build_ksum(kc):
                kps = ps_m.tile([128, QW], F32, tag="ps_misc", name="kps")
                sl = slice(kc * QW, (kc + 1) * QW)
                nc.tensor.matmul(kps[:], wk1[:, 0, :], fmapA[:, 0, sl],
                                 start=True, stop=False)
                nc.tensor.matmul(kps[:], wk1[:, 1, :], fmapA[:, 1, sl],
                                 start=False, stop=False)
                xt = x_tiles[kc]
                for ct in range(16):
                    nc.tensor.matmul(kps[:], wk2[:, ct, :], xt[:, ct, :],
                                     start=False, stop=(ct == 15))
                nc.vector.tensor_copy(ksum[:, sl], kps[:])

            # ---- attention inner step -------------------------------------
            from collections import deque
            pendq = deque()
            ops = {}
            accs = {}

            def attn(qc, kc):
                if kc == 0:
                    ops[qc] = ps_o.tile([128, QW], F32, tag="ps_out",
                                        name=f"ops{qc}")
                    # two-lane denominator accumulator: one flat [128,2,512]
                    # bf16 add per exp tile
                    accs[qc] = accp.tile([128, 2, QW], BF16, tag="dacc",
                                         name=f"acc{qc}")
                ops0 = ops[qc]
                acc = accs[qc]
                for sg in range(KT // 2):
                    sps = ps_s.tile([128, 2, QW], F32, tag="ps_sim")
                    et = exp_pool.tile([128, 2, QW], BF16, tag="exp")
                    for j in range(2):
                        kk = kc * KT + sg * 2 + j
                        nc.tensor.matmul(
                            sps[:, j, :],
                            ksum[:, kk * 128:(kk + 1) * 128], q_sb[:, qc, :],
                            start=True, stop=True)
                    nc.scalar.activation(et[:], sps[:],
                                         mybir.ActivationFunctionType.Exp,
                                         scale=SCALE)
                    if len(pendq) >= 1:
                        pendq.popleft()()

                    first = (kc == 0 and sg == 0)
                    last = (kc == KC - 1 and sg == KT // 2 - 1)

                    def _pend(qc0=qc, kc0=kc, sg0=sg, et0=et, ops0=ops0,
                              acc0=acc, first=first, last=last):
                        for j in range(2):
                            kk = kc0 * KT + sg0 * 2 + j
                            nc.tensor.matmul(ops0[:], vT[:, kk, :],
                                             et0[:, j, :],
                                             start=(first and j == 0),
                                             stop=(last and j == 1))
                        if first:
                            nc.vector.tensor_copy(acc0[:], et0[:])
                        else:
                            nc.vector.tensor_add(acc0[:], acc0[:], et0[:])
                    pendq.append(_pend)

                    if last:
                        # denominator + store finisher runs one pipeline slot
                        # later so the PE never waits on the DVE add chain
                        def _fin(qc0=qc, ops0=ops0, acc0=acc):
                            qsl0 = slice(qc0 * QW, (qc0 + 1) * QW)
                            dps = ps_m.tile([1, QW], F32, tag="ps_misc")
                            nc.tensor.matmul(dps[:], ones[:], acc0[:, 0, :],
                                             start=True, stop=False)
                            nc.tensor.matmul(dps[:], ones[:], acc0[:, 1, :],
                                             start=False, stop=True)
                            den_st = st.tile([1, QW], F32, tag="den_st")
                            nc.vector.tensor_copy(den_st[:], dps[:])
                            nc.sync.dma_start(d_den[:, qsl0], den_st[:])
                            out_st = st.tile([128, QW], F32, tag="out_st")
                            nc.vector.tensor_copy(out_st[:], ops0[:])
                            nc.sync.dma_start(d_outU[:, qsl0], out_st[:])
                        pendq.append(_fin)

            # ---- schedule: ksum built just-in-time with qc 0,1 woven in ----
            build_ksum(0)
            attn(0, 0)
            attn(1, 0)
            build_ksum(1)
            attn(0, 1)
            attn(1, 1)
            build_q(4)
            build_ksum(2)
            attn(0, 2)
            attn(1, 2)
            build_ksum(3)
            attn(0, 3)
            attn(1, 3)
            for qc in range(2, QC):
                # feed q projections into the ACT-gated sweep to keep PE busy
                if qc + 3 < QC:
                    build_q(qc + 3)
                for kc in range(KC):
                    attn(qc, kc)
            while pendq:
                pendq.popleft()()

    nc.compile()
    return nc


def _prep_inputs(fmap, x, Wqkv, Wk2):
    """Host-side slicing: per-core input dicts. Core c = b*4 + h*2 + kh."""
    fmap = np.ascontiguousarray(fmap, dtype=np.float32)
    x = np.ascontiguousarray(x, dtype=np.float32)
    Wqkv = np.ascontiguousarray(Wqkv, dtype=np.float32)
    Wk2 = np.ascontiguousarray(Wk2, dtype=np.float32)

    in_maps = []
    for c in range(8):
        b, h, kh = c // 4, (c // 2) % 2, c % 2
        fb = fmap[b].reshape(C1, N)
        xb = x[b].reshape(C2, N)
        ks = slice(kh * KSL, (kh + 1) * KSL)
        other = slice((1 - kh) * KSL, (2 - kh) * KSL)
        wq = Wqkv[h * D:(h + 1) * D]              # [128, 256]
        wk1 = Wqkv[C1 + h * D:C1 + (h + 1) * D]
        wv = Wqkv[2 * C1 + h * D:2 * C1 + (h + 1) * D]
        wk2 = Wk2[h * D:(h + 1) * D]              # [128, 2048]
        # partition-major layouts: [128, t, n] so DMAs are contiguous per
        # partition; x additionally pre-chunked by 512-key groups
        in_maps.append({
            "fmapA": np.ascontiguousarray(
                fb[:, ks].reshape(2, 128, KSL).transpose(1, 0, 2)).astype(BF16_NP),
            "fmapB": np.ascontiguousarray(
                fb[:, other].reshape(2, 128, KSL).transpose(1, 0, 2)).astype(BF16_NP),
            "xs": np.ascontiguousarray(
                xb[:, ks].reshape(16, 128, KC, QW).transpose(1, 2, 0, 3)).astype(BF16_NP),
            "wqT": np.ascontiguousarray(
                wq.T.reshape(2, 128, D).transpose(1, 0, 2)).astype(BF16_NP),
            "wk1T": np.ascontiguousarray(
                wk1.T.reshape(2, 128, D).transpose(1, 0, 2)).astype(BF16_NP),
            "wvT": np.ascontiguousarray(
                wv.T.reshape(2, 128, D).transpose(1, 0, 2)).astype(BF16_NP),
            "wk2T": np.ascontiguousarray(
                wk2.T.reshape(16, 128, D).transpose(1, 0, 2)).astype(BF16_NP),
        })
    return in_maps


def _combine(results):
    """Host epilogue: un-permute query columns, add key-half partials,
    normalize, assemble output."""
    out = np.empty((2, HEADS * D, 64, 64), dtype=np.float32)
    for b in range(2):
        for h in range(2):
            U = np.empty((D, N), dtype=np.float32)
            Dn = np.empty((1, N), dtype=np.float32)
            for kh in range(2):
                r = results[b * 4 + h * 2 + kh]
                # core kh processed queries in order [kh half, other half]
                cols = np.r_[kh * KSL:(kh + 1) * KSL, (1 - kh) * KSL:(2 - kh) * KSL]
                if kh == 0:
                    U[:, cols] = r["outU"]
                    Dn[:, cols] = r["denom"]
                else:
                    U[:, cols] += r["outU"]
                    Dn[:, cols] += r["denom"]
            out[b, h * D:(h + 1) * D] = (U / Dn).reshape(D, 64, 64)
    return out


def run_on_device(in_maps, trace=False, **kw):
    if "nc" not in _COMPILED:
        _COMPILED["nc"] = _build_program()
    return run_bass_kernel_spmd(_COMPILED["nc"], in_maps, list(range(8)),
                                trace=trace, **kw)


def kernel(fmap, x, Wqkv, Wk2):
    in_maps = _prep_inputs(fmap, x, Wqkv, Wk2)
    res = run_on_device(in_maps)
    return _combine(res.results)


# revision 20
# speedup vs baseline: 1.0644x; 1.0217x over previous
"""Dual-key additive attention (nn_Attention_58059367908033) on 8 trn2 NeuronCores.

Reference computation (per batch b, head h, with n = 64*64 = 4096 positions,
d = 128, scale = d**-0.5):
    q  = Wq_h  @ fmap[b]          # [d, n]   (channels-major, "n" = spatial)
    k1 = Wk1_h @ fmap[b]          # [d, n]
    v  = Wv_h  @ fmap[b]          # [d, n]
    k2 = Wk2_h @ x[b]             # [d, n]
    sim  = (scale * q)^T (k1+k2)  # [n, n]  (q rows, key cols)
    attn = softmax(sim, axis=-1)
    out[b, h*d:(h+1)*d] = (attn @ v^T)^T  # [d, n] -> reshape [d, 64, 64]

Sharding: 8 cores = (b in 2) x (h in 2) x (key-half kh in 2).  Each core
computes, for its (b, h) and its 2048-key slice, the *unnormalized*
    U[d, q]   = sum_{k in slice} exp(scale * sim[k, q]) * vT[k, d]
    D[1, q]   = sum_{k in slice} exp(scale * sim[k, q])
streamed flash-attention style (no max subtraction: |scale*sim| is O(1) for
these inputs, fp32 exp is exact-safe).  The host adds the two key-half
partials and divides -- mathematically exact softmax-attention.

Schedule notes (v2):
  - fmap is sent in two column-halves with the core's KEY half first, so the
    value/k1 projections and early query chunks never wait on the full fmap.
    Queries are processed in that permuted column order; the host swaps the
    output halves back (queries are independent, so this is free).
  - ksum chunks are built just-in-time inside the key-chunk loop, interleaved
    with attention for the first two query chunks, so the 8.4 MB x stream
    overlaps real matmuls instead of warmup filler.
  - Remaining query chunks sweep all key tiles back-to-back, keeping the PE
    dense (HAM stays at K=8/8).
"""

import ml_dtypes
import numpy as np

BF16_NP = ml_dtypes.bfloat16

import concourse.bass as bass
import concourse.mybir as mybir
import concourse.tile as tile
from concourse import bacc
from concourse.bass_utils import run_bass_kernel_spmd

HEADS = 2
D = 128          # dim head
C1 = 256         # fmap channels
C2 = 2048        # x channels
N = 4096         # spatial positions (64*64) = queries = keys
KSL = 2048       # keys per core (half)
SCALE = float(D) ** -0.5

F32 = mybir.dt.float32
BF16 = mybir.dt.bfloat16

KC = 4           # key chunks per core (512 keys each)
KT = 4           # k-tiles (128 keys) per key chunk
QC = 8           # query chunks of 512
QW = 512

_COMPILED = {}


def _build_program():
    nc = bacc.Bacc("TRN2", target_bir_lowering=False, debug=False, num_devices=8)

    # ---- DRAM parameters (per-core data, same program on all 8 cores) ----
    # All tensors are pre-transposed on the host to partition-major layout so
    # every DMA moves multi-KB contiguous runs per partition (no strided
    # small-packet transfers).  fmap arrives in two column halves, the core's
    # key half first; x arrives chunked by 512-key groups.
    d_fmapA = nc.dram_tensor("fmapA", [128, 2, KSL], BF16, kind="ExternalInput").ap()
    d_fmapB = nc.dram_tensor("fmapB", [128, 2, KSL], BF16, kind="ExternalInput").ap()
    d_xs = nc.dram_tensor("xs", [128, KC, 16, QW], BF16, kind="ExternalInput").ap()
    d_wqT = nc.dram_tensor("wqT", [128, 2, D], BF16, kind="ExternalInput").ap()
    d_wk1T = nc.dram_tensor("wk1T", [128, 2, D], BF16, kind="ExternalInput").ap()
    d_wvT = nc.dram_tensor("wvT", [128, 2, D], BF16, kind="ExternalInput").ap()
    d_wk2T = nc.dram_tensor("wk2T", [128, 16, D], BF16, kind="ExternalInput").ap()
    # outputs in the permuted query order (key half first); host un-permutes
    d_outU = nc.dram_tensor("outU", [128, N], F32, kind="ExternalOutput").ap()
    d_den = nc.dram_tensor("denom", [1, N], F32, kind="ExternalOutput").ap()

    with tile.TileContext(nc) as tc:
        with (
            tc.tile_pool(name="wts", bufs=1) as wts,
            tc.tile_pool(name="fm", bufs=1) as fm,
            tc.tile_pool(name="big", bufs=1) as big,
            tc.tile_pool(name="xsp", bufs=1) as xsp,
            tc.tile_pool(name="ex", bufs=6) as exp_pool,
            tc.tile_pool(name="acc", bufs=4) as accp,
            tc.tile_pool(name="st", bufs=2) as st,
            tc.tile_pool(name="ps_s", bufs=2, space="PSUM") as ps_s,
            tc.tile_pool(name="ps_o", bufs=3, space="PSUM") as ps_o,
            tc.tile_pool(name="ps_m", bufs=1, space="PSUM") as ps_m,
        ):
            # ---- input DMAs over the 3 DGE queues ----
            # The two HWDGE queues (sync, scalar) carry the critical-path
            # tensors first: fmapA halves, then x0 halves.  gpsimd (SWDGE)
            # carries the small weights plus the x tail.  Scalar-queue DMAs
            # all issue in the prologue, before the first ACTIVATE.
            fmapA = fm.tile([128, 2, KSL], BF16, tag="fmapA")
            fmapB = fm.tile([128, 2, KSL], BF16, tag="fmapB")

            wv = wts.tile([128, 2, D], BF16, tag="wv")
            wk1 = wts.tile([128, 2, D], BF16, tag="wk1")
            wk2 = wts.tile([128, 16, D], BF16, tag="wk2")
            wq = wts.tile([128, 2, D], BF16, tag="wq")

            x_tiles = [xsp.tile([128, 16, QW], BF16, tag="x", name=f"x{i}",
                                bufs=4)
                       for i in range(KC)]

            def load_x_half(kc, half, eng):
                xt = x_tiles[kc]
                hs = slice(half * 8, half * 8 + 8)
                eng.dma_start(xt[:, hs, :], d_xs[:, kc, hs, :])

            # Deadline-driven spread.  The two HWDGE queues carry the
            # critical path (fmapA halves, weights, x0); the gpsimd queue's
            # x-tail DMAs are GATED behind fmapA / x0 completion via dummy
            # dependency ops so they don't steal HBM bandwidth from the
            # critical transfers (priority inversion).
            nc.sync.dma_start(fmapA[:, 0, :], d_fmapA[:, 0, :])
            nc.scalar.dma_start(fmapA[:, 1, :], d_fmapA[:, 1, :])
            nc.sync.dma_start(wk1[:], d_wk1T)
            nc.sync.dma_start(wv[:], d_wvT)
            nc.scalar.dma_start(wk2[:], d_wk2T)
            nc.scalar.dma_start(wq[:], d_wqT)
            load_x_half(0, 0, nc.sync)
            load_x_half(0, 1, nc.scalar)
            gate = wts.tile([1, 8], BF16, tag="gate")
            nc.gpsimd.tensor_copy(gate[:], x_tiles[0][:1, 0, :8])
            load_x_half(1, 0, nc.gpsimd)
            load_x_half(1, 1, nc.scalar)
            nc.sync.dma_start(fmapB[:], d_fmapB)
            load_x_half(2, 0, nc.gpsimd)
            load_x_half(2, 1, nc.scalar)
            load_x_half(3, 0, nc.gpsimd)
            load_x_half(3, 1, nc.scalar)

            ones = wts.tile([128, 1], BF16, tag="ones")
            nc.vector.memset(ones[:], 1.0)
            warm = wts.tile([128, 512], BF16, tag="warm")
            nc.vector.memset(warm[:], 0.0)

            def warmup(n):
                for _ in range(n):
                    wps = ps_s.tile([128, QW], F32, tag="ps_sim", name="pswarm")
                    nc.tensor.matmul(wps[:], warm[:, :128], warm[:],
                                     start=True, stop=True)

            # HAM ramp: a few no-dep matmuls before real work arrives
            warmup(6)

            # ---- vT tiles [k=128, d] via fmapA-stationary matmuls ----
            vT = big.tile([128, 16, D], BF16, tag="vT")
            for kt in range(16):
                ps = ps_s.tile([128, D], F32, tag="ps_sim", name="psv")
                ksl = slice(kt * 128, (kt + 1) * 128)
                nc.tensor.matmul(ps[:], fmapA[:, 0, ksl], wv[:, 0, :], start=True, stop=False)
                nc.tensor.matmul(ps[:], fmapA[:, 1, ksl], wv[:, 1, :], start=False, stop=True)
                nc.vector.tensor_copy(vT[:, kt, :], ps[:])

            # ---- q projection (permuted order: key half = chunks 0-3) ----
            q_sb = big.tile([128, QC, QW], BF16, tag="q")

            def build_q(nch):
                src = fmapA if nch < 4 else fmapB
                ps = ps_s.tile([128, QW], F32, tag="ps_sim", name="psq")
                sl = slice((nch % 4) * QW, (nch % 4 + 1) * QW)
                nc.tensor.matmul(ps[:], wq[:, 0, :], src[:, 0, sl], start=True, stop=False)
                nc.tensor.matmul(ps[:], wq[:, 1, :], src[:, 1, sl], start=False, stop=True)
                nc.vector.tensor_copy(q_sb[:, nch, :], ps[:])

            for nch in range(4):
                build_q(nch)
            warmup(2)

            ksum = big.tile([128, KSL], BF16, tag="ksum")

            def build_ksum(kc):
                kps = ps_m.tile([128, QW], F32, tag="ps_misc", name="kps")
                sl = slice(kc * QW, (kc + 1) * QW)
                nc.tensor.matmul(kps[:], wk1[:, 0, :], fmapA[:, 0, sl],
                                 start=True, stop=False)
                nc.tensor.matmul(kps[:], wk1[:, 1, :], fmapA[:, 1, sl],
                                 start=False, stop=False)
                xt = x_tiles[kc]
                for ct in range(16):
                    nc.tensor.matmul(kps[:], wk2[:, ct, :], xt[:, ct, :],
                                     start=False, stop=(ct == 15))
                nc.vector.tensor_copy(ksum[:, sl], kps[:])

            # ---- attention inner step -------------------------------------
            from collections import deque
            pendq = deque()
            ops = {}
            accs = {}

            def attn(qc, kc):
                if kc == 0:
                    ops[qc] = ps_o.tile([128, QW], F32, tag="ps_out",
                                        name=f"ops{qc}")
                    # two-lane denominator accumulator: one flat [128,2,512]
                    # bf16 add per exp tile
                    accs[qc] = accp.tile([128, 2, QW], BF16, tag="dacc",
                                         name=f"acc{qc}")
                ops0 = ops[qc]
                acc = accs[qc]
                for sg in range(KT // 2):
                    sps = ps_s.tile([128, 2, QW], F32, tag="ps_sim")
                    et = exp_pool.tile([128, 2, QW], BF16, tag="exp")
                    for j in range(2):
                        kk = kc * KT + sg * 2 + j
                        nc.tensor.matmul(
                            sps[:, j, :],
                            ksum[:, kk * 128:(kk + 1) * 128], q_sb[:, qc, :],
                            start=True, stop=True)
                    nc.scalar.activation(et[:], sps[:],
                                         mybir.ActivationFunctionType.Exp,
                                         scale=SCALE)
                    if len(pendq) >= 1:
                        pendq.popleft()()

                    first = (kc == 0 and sg == 0)
                    last = (kc == KC - 1 and sg == KT // 2 - 1)

                    def _pend(qc0=qc, kc0=kc, sg0=sg, et0=et, ops0=ops0,
                              acc0=acc, first=first, last=last):
                        for j in range(2):
                            kk = kc0 * KT + sg0 * 2 + j
                            nc.tensor.matmul(ops0[:], vT[:, kk, :],
                                             et0[:, j, :],
                                             start=(first and j == 0),
                                             stop=(last and j == 1))
                        if first:
                            nc.vector.tensor_copy(acc0[:], et0[:])
                        else:
                            nc.vector.tensor_add(acc0[:], acc0[:], et0[:])
                    pendq.append(_pend)

                    if last:
                        # denominator + store finisher runs one pipeline slot
                        # later so the PE never waits on the DVE add chain
                        def _fin(qc0=qc, ops0=ops0, acc0=acc):
                            qsl0 = slice(qc0 * QW, (qc0 + 1) * QW)
                            dps = ps_m.tile([1, QW], F32, tag="ps_misc")
                            nc.tensor.matmul(dps[:], ones[:], acc0[:, 0, :],
                                             start=True, stop=False)
                            nc.tensor.matmul(dps[:], ones[:], acc0[:, 1, :],
                                             start=False, stop=True)
                            den_st = st.tile([1, QW], F32, tag="den_st")
                            nc.vector.tensor_copy(den_st[:], dps[:])
                            nc.sync.dma_start(d_den[:, qsl0], den_st[:])
                            out_st = st.tile([128, QW], F32, tag="out_st")
                            nc.vector.tensor_copy(out_st[:], ops0[:])
                            nc.sync.dma_start(d_outU[:, qsl0], out_st[:])
                        pendq.append(_fin)

            # ---- schedule: ksum built just-in-time with qc 0,1 woven in ----
            # Phase A: 3 query chunks woven with the just-in-time ksum builds
            # (all 3 PSUM out banks in use -> less ACT starvation per build).
            for kc in range(KC):
                build_ksum(kc)
                attn(0, kc)
                attn(1, kc)
                if kc == 1:
                    build_q(4)
                attn(2, kc)
            # Phase B: remaining query chunks, ACT-gated; feed the leftover
            # q projections into the PE bubbles.
            for qc in range(3, QC):
                if qc + 2 < QC:
                    build_q(qc + 2)
                for kc in range(KC):
                    attn(qc, kc)
            while pendq:
                pendq.popleft()()

    nc.compile()
    return nc


def _prep_inputs(fmap, x, Wqkv, Wk2):
    """Host-side slicing: per-core input dicts. Core c = b*4 + h*2 + kh."""
    fmap = np.ascontiguousarray(fmap, dtype=np.float32)
    x = np.ascontiguousarray(x, dtype=np.float32)
    Wqkv = np.ascontiguousarray(Wqkv, dtype=np.float32)
    Wk2 = np.ascontiguousarray(Wk2, dtype=np.float32)

    in_maps = []
    for c in range(8):
        b, h, kh = c // 4, (c // 2) % 2, c % 2
        fb = fmap[b].reshape(C1, N)
        xb = x[b].reshape(C2, N)
        ks = slice(kh * KSL, (kh + 1) * KSL)
        other = slice((1 - kh) * KSL, (2 - kh) * KSL)
        wq = Wqkv[h * D:(h + 1) * D]              # [128, 256]
        wk1 = Wqkv[C1 + h * D:C1 + (h + 1) * D]
        wv = Wqkv[2 * C1 + h * D:2 * C1 + (h + 1) * D]
        wk2 = Wk2[h * D:(h + 1) * D]              # [128, 2048]
        # partition-major layouts: [128, t, n] so DMAs are contiguous per
        # partition; x additionally pre-chunked by 512-key groups
        in_maps.append({
            "fmapA": np.ascontiguousarray(
                fb[:, ks].reshape(2, 128, KSL).transpose(1, 0, 2)).astype(BF16_NP),
            "fmapB": np.ascontiguousarray(
                fb[:, other].reshape(2, 128, KSL).transpose(1, 0, 2)).astype(BF16_NP),
            "xs": np.ascontiguousarray(
                xb[:, ks].reshape(16, 128, KC, QW).transpose(1, 2, 0, 3)).astype(BF16_NP),
            "wqT": np.ascontiguousarray(
                wq.T.reshape(2, 128, D).transpose(1, 0, 2)).astype(BF16_NP),
            "wk1T": np.ascontiguousarray(
                wk1.T.reshape(2, 128, D).transpose(1, 0, 2)).astype(BF16_NP),
            "wvT": np.ascontiguousarray(
                wv.T.reshape(2, 128, D).transpose(1, 0, 2)).astype(BF16_NP),
            "wk2T": np.ascontiguousarray(
                wk2.T.reshape(16, 128, D).transpose(1, 0, 2)).astype(BF16_NP),
        })
    return in_maps


def _combine(results):
    """Host epilogue: un-permute query columns, add key-half partials,
    normalize, assemble output."""
    out = np.empty((2, HEADS * D, 64, 64), dtype=np.float32)
    for b in range(2):
        for h in range(2):
            U = np.empty((D, N), dtype=np.float32)
            Dn = np.empty((1, N), dtype=np.float32)
            for kh in range(2):
                r = results[b * 4 + h * 2 + kh]
                # core kh processed queries in order [kh half, other half]
                cols = np.r_[kh * KSL:(kh + 1) * KSL, (1 - kh) * KSL:(2 - kh) * KSL]
                if kh == 0:
                    U[:, cols] = r["outU"]
                    Dn[:, cols] = r["denom"]
                else:
                    U[:, cols] += r["outU"]
                    Dn[:, cols] += r["denom"]
            out[b, h * D:(h + 1) * D] = (U / Dn).reshape(D, 64, 64)
    return out


def run_on_device(in_maps, trace=False, **kw):
    if "nc" not in _COMPILED:
        _COMPILED["nc"] = _build_program()
    return run_bass_kernel_spmd(_COMPILED["nc"], in_maps, list(range(8)),
                                trace=trace, **kw)


def kernel(fmap, x, Wqkv, Wk2):
    in_maps = _prep_inputs(fmap, x, Wqkv, Wk2)
    res = run_on_device(in_maps)
    return _combine(res.results)
